# revision 11
# baseline (speedup 1.0000x reference)
"""Multi-head attention kernel for Trainium2, sharded over 8 NeuronCores.

Sharding: core c handles batch c//4 and heads 4*(c%4) .. 4*(c%4)+4
(data parallel on batch, tensor parallel on heads).  Each core computes a
partial output projection (its heads' slice of Wout); the host sums the 4
partials per batch at gather time.

Self-contained: hardcodes B=2, S=2048, D=1024, H=16.
"""

import numpy as np
import ml_dtypes
from contextlib import ExitStack

import concourse.bass as bass
import concourse.tile as tile
from concourse import mybir
from concourse.bass_utils import run_bass_kernel_spmd

BF16 = ml_dtypes.bfloat16

B, S, D, H = 2, 2048, 1024, 16
A = 64                  # head dim
NH = 4                  # heads per core
M = NH * A              # 256: local projection width
SCALE = 1.0 / 32.0      # 1/sqrt(D)
P = 128
QC = 512                # q chunk (matmul free dim)
NQC = S // QC           # 4
KC = 128                # k chunk (contraction tile for PV)
NKC = S // KC           # 16
DC = D // P             # 8 d-chunks

F32 = mybir.dt.float32
DT16 = mybir.dt.bfloat16
EXP = mybir.ActivationFunctionType.Exp

_prog_cache = {}


def _bcast_part(ap, n):
    """Broadcast a [1, ...] AP across n partitions (step-0 partition dim)."""
    return bass.AP(tensor=ap.tensor, offset=ap.offset, ap=[[0, n]] + list(ap.ap[1:]))


def _build(causal: bool) -> bass.Bass:
    nc = bass.Bass()

    qT = nc.dram_tensor("qT", [D, S], DT16, kind="ExternalInput")
    cT = nc.dram_tensor("cT", [D, S], DT16, kind="ExternalInput")
    wqT = nc.dram_tensor("wqT", [D, M], DT16, kind="ExternalInput")
    wkT = nc.dram_tensor("wkT", [D, M], DT16, kind="ExternalInput")
    wvT = nc.dram_tensor("wvT", [D, M], DT16, kind="ExternalInput")
    woT = nc.dram_tensor("woT", [M, D], DT16, kind="ExternalInput")
    if causal:
        m01 = nc.dram_tensor("m01", [P, 4, 2 * QC], DT16, kind="ExternalInput")
    else:
        emT = nc.dram_tensor("emT", [S, S], DT16, kind="ExternalInput")
    outT = nc.dram_tensor("outT", [D, S], F32, kind="ExternalOutput")

    with tile.TileContext(nc) as tc, ExitStack() as ctx:
        const = ctx.enter_context(tc.tile_pool(name="const", bufs=1))

        # Persistent SBUF tensors
        qt_in = const.tile([P, DC, S], DT16, tag="qt_in")    # query^T  (d on partitions)
        ct_in = const.tile([P, DC, S], DT16, tag="ct_in")    # context^T
        wq_sb = const.tile([P, DC, M], DT16, tag="wq_sb")
        wk_sb = const.tile([P, DC, M], DT16, tag="wk_sb")
        wv_sb = const.tile([P, DC, M], DT16, tag="wv_sb")
        wo_sb = const.tile([P, 2, D], DT16, tag="wo_sb")
        qt = [const.tile([P, S], DT16, tag=f"qt{i}", name=f"qt{i}") for i in range(2)]   # Q^T m-chunks
        kt = [const.tile([P, S], DT16, tag=f"kt{i}", name=f"kt{i}") for i in range(2)]   # K^T m-chunks
        v_sb = const.tile([P, NKC, NH * (A + 1)], DT16, tag="v_sb")       # [V_h | 1] blocks
        u_sb = [const.tile([P, S], DT16, tag=f"u{i}", name=f"u{i}") for i in range(2)]  # normalized attn@V
        if causal:
            m01_sb = const.tile([P, 4, 2 * QC], DT16, tag="m01_sb")
            nc.sync.dma_start(out=m01_sb[:], in_=m01[:, :, :])

        # Input DMAs (d-major chunked so each partition gets contiguous rows)
        qT_r = qT[:, :].rearrange("(c p) s -> p c s", p=P)
        cT_r = cT[:, :].rearrange("(c p) s -> p c s", p=P)
        for dc_ in range(DC):
            nc.sync.dma_start(out=qt_in[:, dc_, :], in_=qT_r[:, dc_, :])
            nc.sync.dma_start(out=ct_in[:, dc_, :], in_=cT_r[:, dc_, :])
        nc.sync.dma_start(out=wq_sb[:], in_=wqT[:, :].rearrange("(c p) m -> p c m", p=P))
        nc.sync.dma_start(out=wk_sb[:], in_=wkT[:, :].rearrange("(c p) m -> p c m", p=P))
        nc.sync.dma_start(out=wv_sb[:], in_=wvT[:, :].rearrange("(c p) m -> p c m", p=P))
        nc.sync.dma_start(out=wo_sb[:], in_=woT[:, :].rearrange("(c p) j -> p c j", p=P))

        nc.vector.memset(v_sb[:], 1.0)  # ones columns for the Z (denominator) trick

        # ---- Projections ----------------------------------------------------
        with tc.tile_pool(name="ps_proj", bufs=3, space="PSUM") as ps_proj:
            for mc in range(2):
                for sc in range(NQC):
                    ps = ps_proj.tile([P, QC], F32, tag="ps_p")
                    for dc_ in range(DC):
                        nc.tensor.matmul(
                            ps[:],
                            lhsT=wq_sb[:, dc_, mc * P:(mc + 1) * P],
                            rhs=qt_in[:, dc_, sc * QC:(sc + 1) * QC],
                            start=(dc_ == 0), stop=(dc_ == DC - 1),
                        )
                    nc.vector.tensor_copy(out=qt[mc][:, sc * QC:(sc + 1) * QC], in_=ps[:])
            for mc in range(2):
                for sc in range(NQC):
                    ps = ps_proj.tile([P, QC], F32, tag="ps_p")
                    for dc_ in range(DC):
                        nc.tensor.matmul(
                            ps[:],
                            lhsT=wk_sb[:, dc_, mc * P:(mc + 1) * P],
                            rhs=ct_in[:, dc_, sc * QC:(sc + 1) * QC],
                            start=(dc_ == 0), stop=(dc_ == DC - 1),
                        )
                    nc.vector.tensor_copy(out=kt[mc][:, sc * QC:(sc + 1) * QC], in_=ps[:])
            # V natural layout [seq, i]; strided copy leaves the ones columns intact
            for cc in range(NKC):
                ps = ps_proj.tile([P, M], F32, tag="ps_v")
                for dc_ in range(DC):
                    nc.tensor.matmul(
                        ps[:],
                        lhsT=ct_in[:, dc_, cc * P:(cc + 1) * P],
                        rhs=wv_sb[:, dc_, :],
                        start=(dc_ == 0), stop=(dc_ == DC - 1),
                    )
                for h in range(NH):
                    nc.vector.tensor_copy(
                        out=v_sb[:, cc, h * (A + 1):h * (A + 1) + A],
                        in_=ps[:, h * A:(h + 1) * A],
                    )

        # ---- Attention ------------------------------------------------------
        with tc.tile_pool(name="ps_s", bufs=2, space="PSUM") as ps_s_pool, \
             tc.tile_pool(name="ps_pv", bufs=2, space="PSUM") as ps_pv_pool, \
             tc.tile_pool(name="expool", bufs=3) as ex_pool, \
             tc.tile_pool(name="zdram", bufs=2, space="DRAM") as zd_pool, \
             tc.tile_pool(name="norm", bufs=2) as norm_pool:
            for pr in range(2):          # head pair
                h0, h1 = 2 * pr, 2 * pr + 1
                for sc in range(NQC):    # query chunk
                    pvA = ps_pv_pool.tile([P, QC], F32, tag="pvA")
                    pvB = ps_pv_pool.tile([P, QC], F32, tag="pvB")
                    nkc = min(4 * sc + 4, NKC) if causal else NKC
                    for kc_ in range(nkc):
                        ps = ps_s_pool.tile([P, 2 * QC], F32, tag="ps_s")
                        # scores^T for both heads of the pair (row groups 0-63 / 64-127)
                        nc.tensor.matmul(
                            ps[:, 0:QC],
                            lhsT=kt[pr][0:A, kc_ * KC:(kc_ + 1) * KC],
                            rhs=qt[pr][0:A, sc * QC:(sc + 1) * QC],
                            start=True, stop=True,
                        )
                        nc.tensor.matmul(
                            ps[:, QC:2 * QC],
                            lhsT=kt[pr][A:2 * A, kc_ * KC:(kc_ + 1) * KC],
                            rhs=qt[pr][A:2 * A, sc * QC:(sc + 1) * QC],
                            start=True, stop=True,
                        )
                        ex = ex_pool.tile([P, 2 * QC], DT16, tag="ex")
                        nc.scalar.activation(out=ex[:], in_=ps[:], func=EXP, scale=SCALE)
                        if causal:
                            r = kc_ - 4 * sc
                            if r >= 0:  # diagonal block: multiplicative 0/1 mask
                                # gpsimd is otherwise idle; keep DVE free
                                nc.gpsimd.tensor_mul(ex[:], ex[:], m01_sb[:, r, :])
                        else:
                            em = ex_pool.tile([P, QC], DT16, tag="em")
                            nc.sync.dma_start(
                                out=em[:],
                                in_=emT[:, :][kc_ * KC:(kc_ + 1) * KC, sc * QC:(sc + 1) * QC],
                            )
                            nc.vector.tensor_mul(ex[:, 0:QC], ex[:, 0:QC], em[:])
                            nc.vector.tensor_mul(ex[:, QC:2 * QC], ex[:, QC:2 * QC], em[:])
                        # PV with ones-column: psum row A holds Z
                        nc.tensor.matmul(
                            pvA[0:A + 1, :],
                            lhsT=v_sb[:, kc_, h0 * (A + 1):(h0 + 1) * (A + 1)],
                            rhs=ex[:, 0:QC],
                            start=(kc_ == 0), stop=(kc_ == nkc - 1),
                        )
                        nc.tensor.matmul(
                            pvB[0:A + 1, :],
                            lhsT=v_sb[:, kc_, h1 * (A + 1):(h1 + 1) * (A + 1)],
                            rhs=ex[:, QC:2 * QC],
                            start=(kc_ == 0), stop=(kc_ == nkc - 1),
                        )
                    # normalize: U / Z.  1/Z lives on one partition; partition-
                    # broadcast needs a DRAM bounce (SBUF APs need nonzero
                    # partition step).
                    zr = norm_pool.tile([P, 2 * QC], F32, tag="zr")
                    nc.vector.reciprocal(out=zr[A:A + 1, 0:QC], in_=pvA[A:A + 1, :])
                    nc.vector.reciprocal(out=zr[A:A + 1, QC:2 * QC], in_=pvB[A:A + 1, :])
                    zd = zd_pool.tile([1, 2 * QC], F32, tag="zd")
                    nc.sync.dma_start(out=zd[:], in_=zr[A:A + 1, :])
                    rb = norm_pool.tile([A, 2 * QC], F32, tag="rb")
                    nc.sync.dma_start(out=rb[:], in_=_bcast_part(zd[0:1, :], A))
                    nc.vector.tensor_mul(
                        u_sb[pr][0:A, sc * QC:(sc + 1) * QC], pvA[0:A, :], rb[:, 0:QC])
                    bt = norm_pool.tile([A, QC], DT16, tag="bt")
                    nc.vector.tensor_mul(bt[:], pvB[0:A, :], rb[:, QC:2 * QC])
                    nc.sync.dma_start(
                        out=u_sb[pr][A:2 * A, sc * QC:(sc + 1) * QC], in_=bt[:])

        # ---- Output projection (partial over local heads) -------------------
        with tc.tile_pool(name="ps_o", bufs=4, space="PSUM") as ps_o, \
             tc.tile_pool(name="o_stage", bufs=4) as o_stage:
            for jc in range(D // P):
                for sc in range(NQC):
                    ps = ps_o.tile([P, QC], F32, tag="ps_o")
                    for ic in range(2):
                        nc.tensor.matmul(
                            ps[:],
                            lhsT=wo_sb[:, ic, jc * P:(jc + 1) * P],
                            rhs=u_sb[ic][:, sc * QC:(sc + 1) * QC],
                            start=(ic == 0), stop=(ic == 1),
                        )
                    o_sb = o_stage.tile([P, QC], F32, tag="o_sb")
                    nc.scalar.copy(out=o_sb[:], in_=ps[:])
                    nc.sync.dma_start(
                        out=outT[:, :][jc * P:(jc + 1) * P, sc * QC:(sc + 1) * QC],
                        in_=o_sb[:])

    return nc


def _split_waits(nc: bass.Bass) -> int:
    """The walrus build here allows one sync wait per engine instruction;
    Tile emits several.  Hoist extras into standalone single-wait
    EventSemaphore instructions on the same engine queue (in-order, so
    semantics are preserved).  DMACopy waits lower into queue descriptors and
    are left alone."""
    n = 0
    for func in nc.m.functions:
        for block in func.blocks:
            out = []
            for ins in block.instructions:
                si = ins.sync_info
                if si is not None and len(si.on_wait) > 1:
                    waits = list(si.on_wait)
                    for w in waits[:-1]:
                        es = mybir.InstEventSemaphore(
                            name=f"waitsplit_{n}", ins=[], outs=[])
                        n += 1
                        es.engine = ins.engine
                        es.sync_info = type(si)(on_wait=[w], on_update=[])
                        out.append(es)
                    si.on_wait = [waits[-1]]
                    ins.sync_info = si
                out.append(ins)
            block.instructions = out
    return n


def _get_prog(causal: bool) -> bass.Bass:
    if causal not in _prog_cache:
        nc = _build(causal)
        _split_waits(nc)
        _prog_cache[causal] = nc
    return _prog_cache[causal]


def _is_causal(mask: np.ndarray) -> bool:
    if mask.shape != (S, S):
        return False
    tri = np.tril(np.ones((S, S), dtype=bool))
    low = mask[tri]
    up = mask[~tri]
    return bool((low == 0.0).all() and (up <= -1e8).all())


def _m01_patterns() -> np.ndarray:
    pats = np.zeros((P, 4, 2 * QC), dtype=BF16)
    f = np.arange(QC)[None, :]
    p = np.arange(P)[:, None]
    for r in range(4):
        pat = (f >= p + KC * r).astype(BF16)
        pats[:, r, 0:QC] = pat
        pats[:, r, QC:2 * QC] = pat
    return pats


def _prep_in_maps(query, context, Wq, Wkv, Wout, mask, causal):
    query = np.asarray(query, dtype=np.float32)
    context = np.asarray(context, dtype=np.float32)
    Wq = np.asarray(Wq, dtype=np.float32)
    Wkv = np.asarray(Wkv, dtype=np.float32)
    Wout = np.asarray(Wout, dtype=np.float32)

    qT = [query[b].T.astype(BF16) for b in range(B)]
    cT = [context[b].T.astype(BF16) for b in range(B)]
    if causal:
        extra = ("m01", _m01_patterns())
    else:
        extra = ("emT", np.exp((SCALE * np.asarray(mask, np.float32).T)).astype(BF16))

    in_maps = []
    for c in range(8):
        b, g = divmod(c, 4)
        m0 = g * M
        in_maps.append({
            "qT": qT[b],
            "cT": cT[b],
            "wqT": Wq[m0:m0 + M, :].T.astype(BF16),
            "wkT": Wkv[m0:m0 + M, :].T.astype(BF16),
            "wvT": Wkv[D + m0:D + m0 + M, :].T.astype(BF16),
            "woT": Wout[:, m0:m0 + M].T.astype(BF16),
            extra[0]: extra[1],
        })
    return in_maps


def _run(query, context, Wq, Wkv, Wout, mask, trace=False):
    causal = _is_causal(np.asarray(mask, np.float32))
    in_maps = _prep_in_maps(query, context, Wq, Wkv, Wout, mask, causal)
    nc = _get_prog(causal)
    res = run_bass_kernel_spmd(nc, in_maps, list(range(8)), trace=trace)
    out = np.zeros((B, S, D), dtype=np.float32)
    for c in range(8):
        out[c // 4] += res.results[c]["outT"].T
    return out, res


def kernel(query, context, Wq, Wkv, Wout, mask):
    out, _ = _run(query, context, Wq, Wkv, Wout, mask, trace=False)
    return out
